# revision 5
# baseline (speedup 1.0000x reference)
"""BERT self-attention (B=4, S=2048, D=1024, H=16) on 8 trn2 NeuronCores.

Sharding: core c -> (batch b = c//2, head-group hg = c%2, 8 heads each).
Each core computes out[b, :, hg*512:(hg+1)*512] independently; host
gathers. Inputs are pre-transposed on host so the contraction dim (d)
lands on SBUF partitions: xt = X.T [D,S], w{q,k,v}t = W.T shard [D,512].

On-device algorithm per core (all matmuls fp32r):
  Q^T, K^T: [o, s] pair-tiles (2 heads / 128 partitions), V: [s, o]
  augmented with a ones column per head (V_aug [s, h, 65]).
  Scores transposed per head: S^T[j, i] = K_h^T.T @ Q_h^T, head pairs
  row-packed on the two PE-array halves (K=64 each).
  U = exp(0.125*S^T + mask[j]) on ACT (mask = per-partition bias).
  ctx_u^T[dh+1, i] = V_aug.T @ U accumulated over j-tiles in PSUM; row 64
  is the softmax denominator (free rowsum via the ones column).
  Final: PE-transpose [65,128] chunks -> [128,65], DVE reciprocal of
  col 64 and tensor_scalar_mul -> out[s, o] tiles -> DMA.
"""

import numpy as np

import concourse.bass as bass
import concourse.tile as tile
from concourse import bacc, mybir
from concourse.bass_utils import run_bass_kernel_spmd
from concourse.masks import make_identity

B, S, D, H = 4, 2048, 1024, 16
DH = 64
O = 512  # per-core output width (8 heads)
HL = 8  # local heads per core
NP = 4  # head pairs per core
ST = S // 128  # 16 s-tiles
F32 = mybir.dt.float32
F32R = mybir.dt.float32r
EXP = mybir.ActivationFunctionType.Exp

_NC_CACHE = None


def build_nc():
    nc = bacc.Bacc(
        "TRN2",
        target_bir_lowering=False,
        debug=False,
        enable_asserts=True,
        num_devices=8,
    )
    xt = nc.dram_tensor("xt", [D, S], F32R, kind="ExternalInput").ap()
    wqt = nc.dram_tensor("wqt", [D, O], F32R, kind="ExternalInput").ap()
    wkt = nc.dram_tensor("wkt", [D, O], F32R, kind="ExternalInput").ap()
    wvt = nc.dram_tensor("wvt", [D, O], F32R, kind="ExternalInput").ap()
    bq = nc.dram_tensor("bq", [O], F32, kind="ExternalInput").ap()
    bk = nc.dram_tensor("bk", [O], F32, kind="ExternalInput").ap()
    bv = nc.dram_tensor("bv", [O], F32, kind="ExternalInput").ap()
    mask = nc.dram_tensor("mask", [S], F32, kind="ExternalInput").ap()
    out = nc.dram_tensor("out", [S, O], F32, kind="ExternalOutput").ap()

    with tile.TileContext(nc) as tc:
        _emit(nc, tc, xt, wqt, wkt, wvt, bq, bk, bv, mask, out)
    nc.compile()
    return nc


def _emit(nc, tc, xt, wqt, wkt, wvt, bq, bk, bv, mask, out):
    with (
        tc.tile_pool(name="singles", bufs=1) as singles,
        tc.tile_pool(name="persist", bufs=1) as persist,
        tc.tile_pool(name="psum", bufs=1, space="PSUM") as psum,
    ):
        ident = singles.tile([128, 128], F32)
        make_identity(nc, ident)
        mask_sb = singles.tile([128, ST], F32)
        nc.sync.dma_start(out=mask_sb, in_=mask.rearrange("(t p) -> p t", p=128))
        bq_sb = singles.tile([128, NP], F32)
        nc.sync.dma_start(out=bq_sb, in_=bq.rearrange("(t p) -> p t", p=128))
        bk_sb = singles.tile([128, NP], F32)
        nc.sync.dma_start(out=bk_sb, in_=bk.rearrange("(t p) -> p t", p=128))
        bv_bc = singles.tile([128, O], F32)
        nc.sync.dma_start(
            out=bv_bc, in_=bass.AP(tensor=bv.tensor, offset=0, ap=[[0, 128], [1, O]])
        )
        ones_sb = singles.tile([128, 1], F32)
        nc.vector.memset(ones_sb, 1.0)

        # persistent activations
        qts = [persist.tile([128, S], F32R, name=f"qt{p}", tag=f"qt{p}") for p in range(NP)]
        kts = [persist.tile([128, S], F32R, name=f"kt{p}", tag=f"kt{p}") for p in range(NP)]
        vaug = [
            persist.tile([128, HL, DH + 1], F32R, name=f"vaug{t}", tag=f"vaug{t}")
            for t in range(ST)
        ]

        stags = ("sA", "sB")

        with tc.tile_pool(name="proj", bufs=1) as proj:
            xts = []
            for dt in range(8):
                xti = proj.tile([128, S], F32R, name=f"xts{dt}", tag=f"xts{dt}")
                nc.sync.dma_start(out=xti, in_=xt[dt * 128 : (dt + 1) * 128, :])
                xts.append(xti)

            def load_w(wdram, label):
                wts = []
                for dt in range(8):
                    w = proj.tile([128, O], F32R, name=f"w{label}{dt}", tag="w", bufs=10)
                    nc.sync.dma_start(out=w, in_=wdram[dt * 128 : (dt + 1) * 128, :])
                    wts.append(w)
                return wts

            k = 0

            def qk_proj(wts, dsts, bias_sb, label):
                nonlocal k
                for p in range(NP):
                    for c in range(4):
                        ps = psum.tile(
                            [128, 512], F32, name=f"pp{label}{p}_{c}", tag=stags[k % 2]
                        )
                        k += 1
                        for dt in range(8):
                            nc.tensor.matmul(
                                ps,
                                wts[dt][:, p * 128 : (p + 1) * 128],
                                xts[dt][:, c * 512 : (c + 1) * 512],
                                start=(dt == 0),
                                stop=(dt == 7),
                            )
                        nc.vector.tensor_scalar_add(
                            dsts[p][:, c * 512 : (c + 1) * 512], ps, bias_sb[:, p : p + 1]
                        )

            wk_t = load_w(wkt, "k")
            qk_proj(wk_t, kts, bk_sb, "k")

            wv_t = load_w(wvt, "v")
            for st in range(ST):
                ps = psum.tile([128, O], F32, name=f"ppv{st}", tag=stags[k % 2])
                k += 1
                for dt in range(8):
                    nc.tensor.matmul(
                        ps,
                        xts[dt][:, st * 128 : (st + 1) * 128],
                        wv_t[dt],
                        start=(dt == 0),
                        stop=(dt == 7),
                    )
                va = vaug[st]
                for h in range(HL):
                    nc.vector.tensor_copy(out=va[:, h, DH : DH + 1], in_=ones_sb)
                for h in range(HL):
                    nc.vector.tensor_add(
                        va[:, h, 0:DH],
                        ps[:, h * DH : (h + 1) * DH],
                        bv_bc[:, h * DH : (h + 1) * DH],
                    )

            wq_t = load_w(wqt, "q")
            qk_proj(wq_t, qts, bq_sb, "q")

        with tc.tile_pool(name="attn", bufs=1) as attn:
            for p in range(NP):
                qtp, ktp = qts[p], kts[p]
                for ih in range(2):
                    cps = [
                        psum.tile([DH + 1, 1024], F32, name=f"ctx{p}_{ih}_{x}", tag=f"ctx{x}")
                        for x in range(2)
                    ]
                    for jt in range(ST):
                        sps = [
                            psum.tile([128, 1024], F32, name=f"s{p}_{ih}_{jt}_{x}", tag=stags[x])
                            for x in range(2)
                        ]
                        for x in range(2):
                            hp = slice(x * 64, x * 64 + 64)
                            for c in range(2):
                                ic = ih * 1024 + c * 512
                                nc.tensor.matmul(
                                    sps[x][:, c * 512 : (c + 1) * 512],
                                    ktp[hp, jt * 128 : (jt + 1) * 128],
                                    qtp[hp, ic : ic + 512],
                                    start=True,
                                    stop=True,
                                )
                        us = []
                        for x in range(2):
                            u = attn.tile(
                                [128, 1024], F32R, name=f"u{p}_{ih}_{jt}_{x}", tag=f"u{x}", bufs=3
                            )
                            nc.scalar.activation(
                                u, sps[x], EXP, bias=mask_sb[:, jt : jt + 1], scale=0.125
                            )
                            us.append(u)
                        for x in range(2):
                            for c in range(2):
                                nc.tensor.matmul(
                                    cps[x][:, c * 512 : (c + 1) * 512],
                                    vaug[jt][:, 2 * p + x, :],
                                    us[x][:, c * 512 : (c + 1) * 512],
                                    start=(jt == 0),
                                    stop=(jt == ST - 1),
                                )
                    # drain: normalize + transpose + store
                    for x in range(2):
                        hh = 2 * p + x
                        cu = attn.tile([DH + 1, 1024], F32, name=f"cu{p}_{ih}_{x}", tag="cu", bufs=2)
                        nc.vector.tensor_copy(out=cu, in_=cps[x])
                        for it in range(8):
                            tp_ = psum.tile(
                                [128, DH + 1], F32, name=f"tp{p}_{ih}_{x}_{it}", tag=stags[x]
                            )
                            nc.tensor.transpose(
                                tp_, cu[:, it * 128 : (it + 1) * 128], ident[0 : DH + 1, 0 : DH + 1]
                            )
                            rc = attn.tile([128, 1], F32, name=f"rc{p}_{ih}_{x}_{it}", tag="rc", bufs=3)
                            nc.vector.reciprocal(rc, tp_[:, DH : DH + 1])
                            ot = attn.tile([128, DH], F32, name=f"ot{p}_{ih}_{x}_{it}", tag="ot", bufs=4)
                            nc.vector.tensor_scalar_mul(ot, tp_[:, 0:DH], rc)
                            row = ih * 1024 + it * 128
                            nc.sync.dma_start(
                                out=out[row : row + 128, hh * DH : (hh + 1) * DH], in_=ot
                            )


def _make_in_maps(hidden_states, attention_mask, Wq, bq, Wk, bk, Wv, bv):
    in_maps = []
    for c in range(8):
        b, hg = divmod(c, 2)
        sl = slice(hg * O, (hg + 1) * O)
        in_maps.append(
            {
                "xt": np.ascontiguousarray(hidden_states[b].T),
                "wqt": np.ascontiguousarray(Wq[sl, :].T),
                "wkt": np.ascontiguousarray(Wk[sl, :].T),
                "wvt": np.ascontiguousarray(Wv[sl, :].T),
                "bq": np.ascontiguousarray(bq[sl]),
                "bk": np.ascontiguousarray(bk[sl]),
                "bv": np.ascontiguousarray(bv[sl]),
                "mask": np.ascontiguousarray(attention_mask[b, 0, 0, :]),
            }
        )
    return in_maps


def _gather(results):
    out = np.empty((B, S, D), dtype=np.float32)
    for c in range(8):
        b, hg = divmod(c, 2)
        out[b, :, hg * O : (hg + 1) * O] = results[c]["out"]
    return out


def kernel(hidden_states, attention_mask, Wq, bq, Wk, bk, Wv, bv, **run_kwargs):
    global _NC_CACHE
    args = [hidden_states, attention_mask, Wq, bq, Wk, bk, Wv, bv]
    args = [np.asarray(a, dtype=np.float32) for a in args]
    if _NC_CACHE is None:
        _NC_CACHE = build_nc()
    in_maps = _make_in_maps(*args)
    res = run_bass_kernel_spmd(_NC_CACHE, in_maps, core_ids=list(range(8)), **run_kwargs)
    kernel.last_result = res
    return _gather(res.results)
